# revision 17
# baseline (speedup 1.0000x reference)
"""Trainium2 Bass kernel for nn_CCepLTVFilter.

Pipeline (all heavy lifting as fixed-matrix matmuls on the PE):
  1. conv1d(x, W) + b            -> ccep_raw[o, bt]        (PE, K=80 x 3 taps)
  2. Yr/Yi = DFT of padded ccep  -> [f, bt]                (PE, lhsT = CF/SF slices)
  3. mag = 10^(Yr/10) via tanh identity; sin/cos(Yi) via ACT Sin
     (single ACT table set; range-wrap on DVE)            -> A, B
  4. Zr/Zi = 1025-point DFT of z frames                   (PE, lhsT = ZC/ZS)
  5. P = (A+iB) * (Zr+iZi)                                (DVE complex mult)
  6. zf = Re(P * e^{-i 2pi f w/1025}) with Hann folded    (PE, lhsT = CO/SO)
  7. overlap-add with circular frame roll                 (DVE)

Sharding: frequency-sharded across 8 cores (f-slice of 128 each); every core
processes all 256 frames; per-core outputs are partial sums of the full
[2,1,32768] output (OLA is linear), summed on gather.
"""

import numpy as np

import concourse.bass as bass
import concourse.bacc as bacc
import concourse.mybir as mybir
import concourse.tile as tile
from concourse.bass_utils import run_bass_kernel_spmd

# ---------------- problem dims (hardcoded) ----------------
B, T, D = 2, 128, 80
CCEP = 222
FFT = 1024
HOP = 256
WIN = 2 * HOP            # 512
PAD = (FFT - CCEP) // 2  # 401
M = FFT + 1              # 1025-point transforms
BT = B * T               # 256
NCORES = 8
FS = FFT // NCORES       # 128 frequencies per core
OC = CCEP // 2           # 111 (o-chunk)
LAM = float(np.log(10.0) / 10.0)

F32 = mybir.dt.float32
PI = float(np.pi)

TRACE = False            # set by test harness for profiling
LAST_RESULT = None       # BassKernelResults of last run (for test harness)


# ---------------- host-side constants (input independent) ----------------
def _make_constants():
    o = np.arange(CCEP, dtype=np.float64)[:, None]
    f = np.arange(FFT, dtype=np.float64)[None, :]
    qn_idx = np.arange(1, CCEP // 2 + 1, dtype=np.float64)
    qnorm = np.concatenate([qn_idx[::-1], qn_idx])
    ang = 2.0 * np.pi * f * (o + PAD) / FFT
    CF = np.cos(ang) * (LAM / 2.0) / qnorm[:, None]      # [222,1024]
    SF = -np.sin(ang) / qnorm[:, None]

    u = np.arange(WIN, dtype=np.float64)[:, None]
    phi = 2.0 * np.pi * f * (u + FFT // 2) / M
    ZC = np.cos(phi)                                     # [512,1024]
    ZS = np.sin(phi)

    w = np.arange(WIN, dtype=np.float64)[None, :]
    th = 2.0 * np.pi * np.arange(FFT, dtype=np.float64)[:, None] * w / M
    win = 0.5 * (1.0 - np.cos(2.0 * np.pi * np.arange(WIN) / WIN))
    CO = np.cos(th) * win[None, :] / M                   # [1024,512]
    SO = np.sin(th) * win[None, :] / M

    consts = []
    for c in range(NCORES):
        sl = slice(c * FS, (c + 1) * FS)
        cfp = CF[:, sl].reshape(2, OC, FS).transpose(1, 0, 2).reshape(OC, 2 * FS)
        sfp = SF[:, sl].reshape(2, OC, FS).transpose(1, 0, 2).reshape(OC, 2 * FS)
        cpack1 = np.concatenate([cfp, sfp], axis=1).astype(np.float32)
        zcp = ZC[:, sl].reshape(4, 128, FS).transpose(1, 0, 2).reshape(128, 4 * FS)
        zsp = ZS[:, sl].reshape(4, 128, FS).transpose(1, 0, 2).reshape(128, 4 * FS)
        cpack2 = np.concatenate(
            [zcp, zsp, CO[sl, :], SO[sl, :]], axis=1).astype(np.float32)
        consts.append(dict(cpack1=np.ascontiguousarray(cpack1),
                           cpack2=np.ascontiguousarray(cpack2)))
    return consts


_CONSTS = _make_constants()
_NC = None


# ---------------- device program ----------------
def _build_nc():
    nc = bacc.Bacc()
    # packed inputs to minimize DMA count (=> few sem-lane waits per consumer)
    sp_e = nc.dram_tensor("spack", [128, 1402], F32, kind="ExternalInput")
    id_e = nc.dram_tensor("ident", [128, 128], F32, kind="ExternalInput")
    c1_e = nc.dram_tensor("cpack1", [OC, 4 * FS], F32, kind="ExternalInput")
    c2_e = nc.dram_tensor("cpack2", [128, 16 * FS], F32, kind="ExternalInput")
    zp_e = nc.dram_tensor("zpad", [B, HOP + T * HOP], F32, kind="ExternalInput")
    out_e = nc.dram_tensor("out", [B, 1, T * HOP], F32, kind="ExternalOutput")

    with tile.TileContext(nc) as tc:
        with tc.tile_pool(name="sb", bufs=1) as sb, \
             tc.tile_pool(name="ps", bufs=2, space="PSUM") as ps:

            # ---- input DMAs (few, large) ----
            spack = sb.tile([128, 1402], F32, tag="spack", name="spack")
            nc.sync.dma_start(out=spack[:], in_=sp_e[:, :])
            xT = spack[0:D, 0:BT]
            wt = spack[0:D, BT:BT + 3 * CCEP]
            bias_row = spack[0:1, 924:924 + CCEP]
            ones_row = spack[0:1, 1146:1402]
            ident_t = sb.tile([128, 128], F32, tag="ident", name="ident_t")
            nc.sync.dma_start(out=ident_t[:], in_=id_e[:, :])
            ident = ident_t[:, :]
            cp1 = sb.tile([OC, 4 * FS], F32, tag="cp1", name="cp1")
            nc.sync.dma_start(out=cp1[:], in_=c1_e[:, :])
            cf = cp1[:, 0:2 * FS]
            sf = cp1[:, 2 * FS:4 * FS]
            cp2 = sb.tile([128, 16 * FS], F32, tag="cp2", name="cp2")
            nc.sync.dma_start(out=cp2[:], in_=c2_e[:, :])
            zc = cp2[:, 0:4 * FS]
            zs = cp2[:, 4 * FS:8 * FS]
            co = cp2[:, 8 * FS:12 * FS]
            so = cp2[:, 12 * FS:16 * FS]

            # frames natural layout [t, (b,u)]: frames[b,t,u] = zpad[b, t*HOP+u]
            fnat = sb.tile([T, B * WIN], F32, tag="fnat", name="fnat")
            for bb in range(B):
                src = bass.AP(zp_e[:, :].tensor, bb * (HOP + T * HOP),
                              [[HOP, T], [1, WIN]])
                nc.sync.dma_start(
                    out=fnat[:, bb * WIN:(bb + 1) * WIN], in_=src)

            # transpose to frames^T [u, (chunk b t)] via PE
            fr = sb.tile([128, 4 * BT], F32, tag="frames")
            for mc in range(4):
                for bb in range(B):
                    tp = ps.tile([128, T], F32, tag="tpA", bufs=2, name=f"ftp{mc}{bb}")
                    nc.tensor.transpose(
                        tp[:, :], fnat[:, bb * WIN + mc * 128: bb * WIN + (mc + 1) * 128],
                        ident)
                    nc.scalar.copy(
                        fr[:, mc * BT + bb * T: mc * BT + (bb + 1) * T], tp[:, :])

            # ---- conv: ccep_raw[o, bt] = sum_k Wk @ x_shift(k) + bias ----
            ccep = []
            for c in range(2):
                pc = ps.tile([OC, BT], F32, tag="tpB", bufs=2, name=f"conv{c}")
                wk = lambda k: spack[0:D, BT + k * CCEP + c * OC:
                                     BT + k * CCEP + c * OC + OC]
                # bias as rank-1 matmul (covers all columns, sets has_written)
                nc.tensor.matmul(pc[:, :], bias_row[:, c * OC:(c + 1) * OC],
                                 ones_row, start=True, stop=False)
                nc.tensor.matmul(pc[:, :], wk(1), xT[:, :], start=False,
                                 stop=False)
                for bb in range(B):
                    lo = bb * T
                    # k=0: out cols t>=1 from x cols t-1
                    nc.tensor.matmul(pc[:, lo + 1: lo + T], wk(0),
                                     xT[:, lo: lo + T - 1], start=False, stop=False)
                    # k=2: out cols t<T-1 from x cols t+1
                    last = (bb == B - 1)
                    nc.tensor.matmul(pc[:, lo: lo + T - 1], wk(2),
                                     xT[:, lo + 1: lo + T], start=False, stop=last)
                cs = sb.tile([OC, BT], F32, tag=f"ccep{c}", name=f"ccep{c}")
                nc.scalar.copy(cs[:, :], pc[:, :])
                ccep.append(cs)

            # ---- step2: Yr/Yi [f_local, bt] ----
            yr = ps.tile([FS, BT], F32, tag="tpC", bufs=4, name="yr")
            yi = ps.tile([FS, BT], F32, tag="tpC", bufs=4, name="yi")
            for c in range(2):
                nc.tensor.matmul(yr[:, :], cf[:, c * FS:(c + 1) * FS], ccep[c][:, :],
                                 start=(c == 0), stop=(c == 1))
            for c in range(2):
                nc.tensor.matmul(yi[:, :], sf[:, c * FS:(c + 1) * FS], ccep[c][:, :],
                                 start=(c == 0), stop=(c == 1))

            # ---- step3: mag, sin, cos -> A, B ----
            def wtile(name):
                return sb.tile([FS, BT], F32, tag=name, name=name)

            # range-reduce Yi into [-pi,pi] first so ACT sin/cos start early
            yiw = wtile("yiw")
            nc.vector.add_range_wrap(yiw[:, :], yi[:, :], 0.0, PI, 2.0 * PI)
            yic = wtile("yic")
            nc.vector.add_range_wrap(yic[:, :], yi[:, :], PI / 2.0, PI, 2.0 * PI)
            sinv = wtile("sinv")
            nc.scalar.activation(sinv[:, :], yiw[:, :],
                                 mybir.ActivationFunctionType.Sin)
            cosv = wtile("cosv")
            nc.scalar.activation(cosv[:, :], yic[:, :],
                                 mybir.ActivationFunctionType.Sin)
            # mag = 10^(Yr/10) = (1+t)/(1-t), t = tanh(Yr * ln10/20) (scale
            # folded into CF) -- stays in the same ACT table set as Sin
            th = wtile("th")
            nc.scalar.activation(th[:, :], yr[:, :],
                                 mybir.ActivationFunctionType.Tanh)
            num = wtile("num")
            nc.vector.tensor_scalar(num[:, :], th[:, :], 1.0, None,
                                    mybir.AluOpType.add)
            den = wtile("den")
            nc.vector.tensor_scalar(den[:, :], th[:, :], -1.0, 1.0,
                                    mybir.AluOpType.mult, mybir.AluOpType.add)
            rscr = wtile("rscr")
            rcp = wtile("rcp")
            nc.vector.reciprocal_approx_accurate(rcp[:, :], den[:, :], rscr[:, :])
            mag = wtile("mag")
            nc.vector.tensor_tensor(mag[:, :], num[:, :], rcp[:, :],
                                    mybir.AluOpType.mult)
            Av = wtile("Av")
            nc.vector.tensor_tensor(Av[:, :], mag[:, :], cosv[:, :],
                                    mybir.AluOpType.mult)
            Bv = wtile("Bv")
            nc.vector.tensor_tensor(Bv[:, :], mag[:, :], sinv[:, :],
                                    mybir.AluOpType.mult)

            # ---- step4: Zr/Zi [f_local, bt] ----
            zr = ps.tile([FS, BT], F32, tag="tpC", bufs=4, name="zr")
            zi = ps.tile([FS, BT], F32, tag="tpC", bufs=4, name="zi")
            for mc in range(4):
                nc.tensor.matmul(zr[:, :], zc[:, mc * FS:(mc + 1) * FS],
                                 fr[:, mc * BT:(mc + 1) * BT],
                                 start=(mc == 0), stop=(mc == 3))
            for mc in range(4):
                nc.tensor.matmul(zi[:, :], zs[:, mc * FS:(mc + 1) * FS],
                                 fr[:, mc * BT:(mc + 1) * BT],
                                 start=(mc == 0), stop=(mc == 3))

            # ---- step5: P = (A + iB)(Zr + iZi) ----
            t1 = wtile("t1")
            nc.vector.tensor_tensor(t1[:, :], Av[:, :], zr[:, :],
                                    mybir.AluOpType.mult)
            t2 = wtile("t2")
            nc.vector.tensor_tensor(t2[:, :], Bv[:, :], zi[:, :],
                                    mybir.AluOpType.mult)
            Pr = wtile("Pr")
            nc.vector.tensor_tensor(Pr[:, :], t1[:, :], t2[:, :],
                                    mybir.AluOpType.subtract)
            t3 = wtile("t3")
            nc.vector.tensor_tensor(t3[:, :], Av[:, :], zi[:, :],
                                    mybir.AluOpType.mult)
            t4 = wtile("t4")
            nc.vector.tensor_tensor(t4[:, :], Bv[:, :], zr[:, :],
                                    mybir.AluOpType.mult)
            Pi = wtile("Pi")
            nc.vector.tensor_tensor(Pi[:, :], t3[:, :], t4[:, :],
                                    mybir.AluOpType.add)

            # ---- step6: zf[w, bt] partial (Hann + 1/1025 folded into CO/SO) ----
            zf = []
            for wti in range(4):
                zt = ps.tile([128, BT], F32, tag="tpB" if wti < 2 else "tpA",
                             bufs=2, name=f"zf{wti}")
                nc.tensor.matmul(zt[:, :], co[:, wti * 128:(wti + 1) * 128],
                                 Pr[:, :], start=True, stop=False)
                nc.tensor.matmul(zt[:, :], so[:, wti * 128:(wti + 1) * 128],
                                 Pi[:, :], start=False, stop=True)
                zf.append(zt)

            # ---- OLA: o[w', (b,t)] = zf[w',(b,t)] + zf[w'+256,(b,(t-1)%T)] ----
            s2 = wtile("s2")
            nc.vector.tensor_copy(s2[:, :], zf[2][:, :])
            s3 = wtile("s3")
            nc.vector.tensor_copy(s3[:, :], zf[3][:, :])
            outs = []
            for j in range(2):
                oj = sb.tile([128, BT], F32, tag=f"o{j}")
                lv = zf[j].rearrange("p (b t) -> p b t", b=B)
                rv = (s2 if j == 0 else s3).rearrange("p (b t) -> p b t", b=B)
                ov = oj.rearrange("p (b t) -> p b t", b=B)
                nc.vector.tensor_tensor(ov[:, :, 1:T], lv[:, :, 1:T],
                                        rv[:, :, 0:T - 1], mybir.AluOpType.add)
                nc.vector.tensor_tensor(ov[:, :, 0:1], lv[:, :, 0:1],
                                        rv[:, :, T - 1:T], mybir.AluOpType.add)
                outs.append(oj)

            # ---- output: transpose to [t, w'] then store contiguous runs ----
            # out[b, 0, t*256 + j*128 + p] = o_j[p, (b,t)]
            for j in range(2):
                for bb in range(B):
                    ot = ps.tile([T, 128], F32, tag="tpC", bufs=4, name=f"ot{j}{bb}")
                    nc.tensor.transpose(
                        ot[:, :], outs[j][:, bb * T:(bb + 1) * T], ident)
                    os_ = sb.tile([T, 128], F32, tag=f"os{j}{bb}",
                                  name=f"os{j}{bb}")
                    nc.scalar.copy(os_[:, :], ot[:, :])
                    dst = bass.AP(out_e[:, :, :].tensor,
                                  bb * T * HOP + j * 128, [[HOP, T], [1, 128]])
                    nc.sync.dma_start(out=dst, in_=os_[:, :])

    return nc


def _get_nc():
    global _NC
    if _NC is None:
        _NC = _build_nc()
        _NC.finalize()
    return _NC


# ---------------- host orchestration ----------------
def kernel(x, z, W, b):
    global LAST_RESULT
    x = np.ascontiguousarray(np.asarray(x, dtype=np.float32))
    z = np.ascontiguousarray(np.asarray(z, dtype=np.float32))
    W = np.ascontiguousarray(np.asarray(W, dtype=np.float32))
    b = np.ascontiguousarray(np.asarray(b, dtype=np.float32))

    xT = np.ascontiguousarray(x.reshape(BT, D).T)                 # [80, 256]
    wt = np.concatenate([W[:, :, k].T for k in range(3)], axis=1)  # [80, 666]
    spack = np.zeros((128, 1402), np.float32)
    spack[0:D, 0:BT] = xT
    spack[0:D, BT:BT + 3 * CCEP] = wt
    spack[0, 924:924 + CCEP] = b
    spack[0, 1146:1402] = 1.0
    zpad = np.concatenate(
        [np.zeros((B, HOP), np.float32), z[:, 0, :]], axis=1)     # [2, 33024]
    shared = {"spack": spack, "zpad": zpad,
              "ident": np.eye(128, dtype=np.float32)}
    in_maps = [{**shared, **_CONSTS[c]} for c in range(NCORES)]

    nc = _get_nc()
    res = run_bass_kernel_spmd(nc, in_maps, list(range(NCORES)), trace=TRACE)
    LAST_RESULT = res
    out = np.zeros((B, 1, T * HOP), dtype=np.float32)
    for r in res.results:
        out += np.asarray(r["out"], dtype=np.float32)
    return out


# revision 22
# speedup vs baseline: 1.1671x; 1.1671x over previous
"""Trainium2 Bass kernel for nn_CCepLTVFilter.

Pipeline (all heavy lifting as fixed-matrix matmuls on the PE):
  1. conv1d(x, W) + b            -> ccep_raw[o, bt]        (PE, K=80 x 3 taps)
  2. Yr/Yi = DFT of padded ccep  -> [f, bt]                (PE, lhsT = CF/SF slices)
  3. mag = 10^(Yr/10) via tanh identity; sin/cos(Yi) via ACT Sin
     (single ACT table set; range-wrap on DVE)            -> A, B
  4. Zr/Zi = 1025-point DFT of z frames                   (PE, lhsT = ZC/ZS)
  5. P = (A+iB) * (Zr+iZi)                                (DVE complex mult)
  6. zf = Re(P * e^{-i 2pi f w/1025}) with Hann folded    (PE, lhsT = CO/SO)
  7. overlap-add with circular frame roll                 (DVE)

Sharding: frequency-sharded across 8 cores (f-slice of 128 each); every core
processes all 256 frames; per-core outputs are partial sums of the full
[2,1,32768] output (OLA is linear), summed on gather.
"""

import numpy as np

import concourse.bass as bass
import concourse.bacc as bacc
import concourse.mybir as mybir
import concourse.tile as tile
from concourse.bass_utils import run_bass_kernel_spmd

# ---------------- problem dims (hardcoded) ----------------
B, T, D = 2, 128, 80
CCEP = 222
FFT = 1024
HOP = 256
WIN = 2 * HOP            # 512
PAD = (FFT - CCEP) // 2  # 401
M = FFT + 1              # 1025-point transforms
BT = B * T               # 256
NCORES = 8
FS = FFT // NCORES       # 128 frequencies per core
OC = CCEP // 2           # 111 (o-chunk)
LAM = float(np.log(10.0) / 10.0)

F32 = mybir.dt.float32
F32R = mybir.dt.float32r
PI = float(np.pi)
USE_F32R = True


def _r(ap):
    return ap

TRACE = False            # set by test harness for profiling
LAST_RESULT = None       # BassKernelResults of last run (for test harness)


# ---------------- host-side constants (input independent) ----------------
def _make_constants():
    o = np.arange(CCEP, dtype=np.float64)[:, None]
    f = np.arange(FFT, dtype=np.float64)[None, :]
    qn_idx = np.arange(1, CCEP // 2 + 1, dtype=np.float64)
    qnorm = np.concatenate([qn_idx[::-1], qn_idx])
    ang = 2.0 * np.pi * f * (o + PAD) / FFT
    CF = np.cos(ang) * (LAM / 2.0) / qnorm[:, None]      # [222,1024]
    SF = -np.sin(ang) / qnorm[:, None]

    u = np.arange(WIN, dtype=np.float64)[:, None]
    phi = 2.0 * np.pi * f * (u + FFT // 2) / M
    ZC = np.cos(phi)                                     # [512,1024]
    ZS = np.sin(phi)

    w = np.arange(WIN, dtype=np.float64)[None, :]
    th = 2.0 * np.pi * np.arange(FFT, dtype=np.float64)[:, None] * w / M
    win = 0.5 * (1.0 - np.cos(2.0 * np.pi * np.arange(WIN) / WIN))
    CO = np.cos(th) * win[None, :] / M                   # [1024,512]
    SO = np.sin(th) * win[None, :] / M

    consts = []
    for c in range(NCORES):
        sl = slice(c * FS, (c + 1) * FS)
        cfp = CF[:, sl].reshape(2, OC, FS).transpose(1, 0, 2).reshape(OC, 2 * FS)
        sfp = SF[:, sl].reshape(2, OC, FS).transpose(1, 0, 2).reshape(OC, 2 * FS)
        cpack1 = np.concatenate([cfp, sfp], axis=1).astype(np.float32)
        zcp = ZC[:, sl].reshape(4, 128, FS).transpose(1, 0, 2).reshape(128, 4 * FS)
        zsp = ZS[:, sl].reshape(4, 128, FS).transpose(1, 0, 2).reshape(128, 4 * FS)
        cpack2 = np.concatenate(
            [zcp, zsp, CO[sl, :], SO[sl, :]], axis=1).astype(np.float32)
        consts.append(dict(cpack1=np.ascontiguousarray(cpack1),
                           cpack2=np.ascontiguousarray(cpack2)))
    return consts


_CONSTS = _make_constants()
_NC = None


# ---------------- device program ----------------
def _build_nc():
    nc = bacc.Bacc()
    # packed inputs to minimize DMA count (=> few sem-lane waits per consumer)
    sp_e = nc.dram_tensor("spack", [128, 956], F32, kind="ExternalInput")
    id_e = nc.dram_tensor("ident", [128, 128], F32, kind="ExternalInput")
    c1_e = nc.dram_tensor("cpack1", [OC, 4 * FS], F32, kind="ExternalInput")
    c2_e = nc.dram_tensor("cpack2", [128, 16 * FS], F32, kind="ExternalInput")
    zp_e = nc.dram_tensor("zpad", [B, HOP + T * HOP], F32, kind="ExternalInput")
    out_e = nc.dram_tensor("out", [B, 1, T * HOP], F32, kind="ExternalOutput")

    with tile.TileContext(nc) as tc:
        with tc.tile_pool(name="sb", bufs=1) as sb, \
             tc.tile_pool(name="ps", bufs=2, space="PSUM") as ps:

            # ---- input DMAs (few, large) ----
            spack = sb.tile([128, 956], F32R, tag="spack", name="spack")
            nc.gpsimd.dma_start(out=spack[:], in_=sp_e[:, :])
            # xcat rows = (k*80+d) shifted x + ones row (241 rows in 2 chunks)
            xcatA = spack[0:121, 0:BT]
            xcatB = spack[0:120, BT:2 * BT]
            w2A = spack[0:121, 2 * BT:2 * BT + CCEP]
            w2B = spack[0:120, 2 * BT + CCEP:2 * BT + 2 * CCEP]
            ident_t = sb.tile([128, 128], F32, tag="ident", name="ident_t")
            nc.sync.dma_start(out=ident_t[:], in_=id_e[:, :])
            ident = ident_t[:, :]
            cp1 = sb.tile([OC, 4 * FS], F32R, tag="cp1", name="cp1")
            nc.gpsimd.dma_start(out=cp1[:], in_=c1_e[:, :])
            cf = cp1[:, 0:2 * FS]
            sf = cp1[:, 2 * FS:4 * FS]
            cp2 = sb.tile([128, 16 * FS], F32R, tag="cp2", name="cp2")
            nc.gpsimd.dma_start(out=cp2[:], in_=c2_e[:, :])
            zc = cp2[:, 0:4 * FS]
            zs = cp2[:, 4 * FS:8 * FS]
            co = cp2[:, 8 * FS:12 * FS]
            so = cp2[:, 12 * FS:16 * FS]

            # frames natural layout [t, (b,u)]: frames[b,t,u] = zpad[b, t*HOP+u]
            fnat = sb.tile([T, B * WIN], F32, tag="fnat", name="fnat")
            for bb in range(B):
                src = bass.AP(zp_e[:, :].tensor, bb * (HOP + T * HOP),
                              [[HOP, T], [1, WIN]])
                nc.sync.dma_start(
                    out=fnat[:, bb * WIN:(bb + 1) * WIN], in_=src)

            # transpose to frames^T [u, (chunk b t)] via PE
            fr = sb.tile([128, 4 * BT], F32R, tag="frames")
            for mc in range(4):
                for bb in range(B):
                    tp = ps.tile([128, T], F32, tag="tpA", bufs=2, name=f"ftp{mc}{bb}")
                    nc.tensor.transpose(
                        tp[:, :], fnat[:, bb * WIN + mc * 128: bb * WIN + (mc + 1) * 128],
                        ident)
                    nc.scalar.copy(
                        fr[:, mc * BT + bb * T: mc * BT + (bb + 1) * T], tp[:, :])

            # ---- conv: ccep_raw[o, bt] = W2.T @ xcat (bias via ones row) ----
            ccep = []
            for c in range(2):
                pc = ps.tile([OC, BT], F32, tag="tpB", bufs=2, name=f"conv{c}")
                nc.tensor.matmul(pc[:, :], w2A[:, c * OC:(c + 1) * OC],
                                 xcatA, start=True, stop=False)
                nc.tensor.matmul(pc[:, :], w2B[:, c * OC:(c + 1) * OC],
                                 xcatB, start=False, stop=True)
                cs = sb.tile([OC, BT], F32R, tag=f"ccep{c}", name=f"ccep{c}")
                nc.scalar.copy(cs[:, :], pc[:, :])
                ccep.append(cs)

            # ---- step2: Yr/Yi [f_local, bt] ----
            yr = ps.tile([FS, BT], F32, tag="tpC", bufs=4, name="yr")
            yi = ps.tile([FS, BT], F32, tag="tpC", bufs=4, name="yi")
            for c in range(2):
                nc.tensor.matmul(yr[:, :], _r(cf[:, c * FS:(c + 1) * FS]),
                                 _r(ccep[c][:, :]),
                                 start=(c == 0), stop=(c == 1))
            for c in range(2):
                nc.tensor.matmul(yi[:, :], _r(sf[:, c * FS:(c + 1) * FS]),
                                 _r(ccep[c][:, :]),
                                 start=(c == 0), stop=(c == 1))

            # ---- step3: mag, sin, cos -> A, B ----
            def wtile(name):
                return sb.tile([FS, BT], F32, tag=name, name=name)

            # range-reduce Yi into [-pi,pi] first so ACT sin/cos start early
            yiw = wtile("yiw")
            nc.vector.add_range_wrap(yiw[:, :], yi[:, :], 0.0, PI, 2.0 * PI)
            yic = wtile("yic")
            nc.vector.add_range_wrap(yic[:, :], yi[:, :], PI / 2.0, PI, 2.0 * PI)
            sinv = wtile("sinv")
            nc.scalar.activation(sinv[:, :], yiw[:, :],
                                 mybir.ActivationFunctionType.Sin)
            cosv = wtile("cosv")
            nc.scalar.activation(cosv[:, :], yic[:, :],
                                 mybir.ActivationFunctionType.Sin)
            # mag = 10^(Yr/10) = (1+t)/(1-t), t = tanh(Yr * ln10/20) (scale
            # folded into CF) -- stays in the same ACT table set as Sin
            th = wtile("th")
            nc.scalar.activation(th[:, :], yr[:, :],
                                 mybir.ActivationFunctionType.Tanh)
            num = wtile("num")
            nc.vector.tensor_scalar(num[:, :], th[:, :], 1.0, None,
                                    mybir.AluOpType.add)
            den = wtile("den")
            nc.vector.tensor_scalar(den[:, :], th[:, :], -1.0, 1.0,
                                    mybir.AluOpType.mult, mybir.AluOpType.add)
            rscr = wtile("rscr")
            rcp = wtile("rcp")
            nc.vector.reciprocal_approx_accurate(rcp[:, :], den[:, :], rscr[:, :])
            mag = wtile("mag")
            nc.vector.tensor_tensor(mag[:, :], num[:, :], rcp[:, :],
                                    mybir.AluOpType.mult)
            Av = wtile("Av")
            nc.vector.tensor_tensor(Av[:, :], mag[:, :], cosv[:, :],
                                    mybir.AluOpType.mult)
            Bv = wtile("Bv")
            nc.vector.tensor_tensor(Bv[:, :], mag[:, :], sinv[:, :],
                                    mybir.AluOpType.mult)

            # ---- step4: Zr/Zi [f_local, bt] ----
            zr = ps.tile([FS, BT], F32, tag="tpC", bufs=4, name="zr")
            zi = ps.tile([FS, BT], F32, tag="tpC", bufs=4, name="zi")
            for mc in range(4):
                nc.tensor.matmul(zr[:, :], _r(zc[:, mc * FS:(mc + 1) * FS]),
                                 _r(fr[:, mc * BT:(mc + 1) * BT]),
                                 start=(mc == 0), stop=(mc == 3))
            for mc in range(4):
                nc.tensor.matmul(zi[:, :], _r(zs[:, mc * FS:(mc + 1) * FS]),
                                 _r(fr[:, mc * BT:(mc + 1) * BT]),
                                 start=(mc == 0), stop=(mc == 3))

            # ---- step5: P = (A + iB)(Zr + iZi) ----
            t1 = wtile("t1")
            nc.vector.tensor_tensor(t1[:, :], Av[:, :], zr[:, :],
                                    mybir.AluOpType.mult)
            t2 = wtile("t2")
            nc.vector.tensor_tensor(t2[:, :], Bv[:, :], zi[:, :],
                                    mybir.AluOpType.mult)
            Pr = sb.tile([FS, BT], F32R, tag="Pr", name="Pr")
            nc.vector.tensor_tensor(Pr[:, :], t1[:, :], t2[:, :],
                                    mybir.AluOpType.subtract)
            t3 = wtile("t3")
            nc.vector.tensor_tensor(t3[:, :], Av[:, :], zi[:, :],
                                    mybir.AluOpType.mult)
            t4 = wtile("t4")
            nc.vector.tensor_tensor(t4[:, :], Bv[:, :], zr[:, :],
                                    mybir.AluOpType.mult)
            Pi = sb.tile([FS, BT], F32R, tag="Pi", name="Pi")
            nc.vector.tensor_tensor(Pi[:, :], t3[:, :], t4[:, :],
                                    mybir.AluOpType.add)

            # ---- step6: zf[w, bt] partial (Hann + 1/1025 folded into CO/SO) ----
            zf = []
            for wti in range(4):
                zt = ps.tile([128, BT], F32, tag="tpB" if wti < 2 else "tpA",
                             bufs=2, name=f"zf{wti}")
                nc.tensor.matmul(zt[:, :], _r(co[:, wti * 128:(wti + 1) * 128]),
                                 _r(Pr[:, :]), start=True, stop=False)
                nc.tensor.matmul(zt[:, :], _r(so[:, wti * 128:(wti + 1) * 128]),
                                 _r(Pi[:, :]), start=False, stop=True)
                zf.append(zt)

            # ---- OLA: o[w', (b,t)] = zf[w',(b,t)] + zf[w'+256,(b,(t-1)%T)] ----
            s2 = wtile("s2")
            nc.vector.tensor_copy(s2[:, :], zf[2][:, :])
            s3 = wtile("s3")
            nc.vector.tensor_copy(s3[:, :], zf[3][:, :])
            outs = []
            for j in range(2):
                oj = sb.tile([128, BT], F32, tag=f"o{j}")
                lv = zf[j].rearrange("p (b t) -> p b t", b=B)
                rv = (s2 if j == 0 else s3).rearrange("p (b t) -> p b t", b=B)
                ov = oj.rearrange("p (b t) -> p b t", b=B)
                nc.vector.tensor_tensor(ov[:, :, 1:T], lv[:, :, 1:T],
                                        rv[:, :, 0:T - 1], mybir.AluOpType.add)
                nc.vector.tensor_tensor(ov[:, :, 0:1], lv[:, :, 0:1],
                                        rv[:, :, T - 1:T], mybir.AluOpType.add)
                outs.append(oj)

            # ---- output: transpose to [t, w'] then store contiguous runs ----
            # out[b, 0, t*256 + j*128 + p] = o_j[p, (b,t)]
            for j in range(2):
                for bb in range(B):
                    ot = ps.tile([T, 128], F32, tag="tpC", bufs=4, name=f"ot{j}{bb}")
                    nc.tensor.transpose(
                        ot[:, :], outs[j][:, bb * T:(bb + 1) * T], ident)
                    os_ = sb.tile([T, 128], F32, tag=f"os{j}{bb}",
                                  name=f"os{j}{bb}")
                    nc.scalar.copy(os_[:, :], ot[:, :])
                    dst = bass.AP(out_e[:, :, :].tensor,
                                  bb * T * HOP + j * 128, [[HOP, T], [1, 128]])
                    nc.sync.dma_start(out=dst, in_=os_[:, :])

    return nc


def _get_nc():
    global _NC
    if _NC is None:
        _NC = _build_nc()
        _NC.finalize()
    return _NC


# ---------------- host orchestration ----------------
def kernel(x, z, W, b):
    global LAST_RESULT
    x = np.ascontiguousarray(np.asarray(x, dtype=np.float32))
    z = np.ascontiguousarray(np.asarray(z, dtype=np.float32))
    W = np.ascontiguousarray(np.asarray(W, dtype=np.float32))
    b = np.ascontiguousarray(np.asarray(b, dtype=np.float32))

    xT = np.ascontiguousarray(x.reshape(BT, D).T)                 # [80, 256]
    xsh = np.zeros((3, D, BT), np.float32)
    xsh[1] = xT
    xv = xT.reshape(D, B, T)
    xsh[0].reshape(D, B, T)[:, :, 1:] = xv[:, :, :-1]
    xsh[2].reshape(D, B, T)[:, :, :-1] = xv[:, :, 1:]
    xcat = np.concatenate([xsh.reshape(3 * D, BT),
                           np.ones((1, BT), np.float32)], axis=0)  # [241,256]
    w2 = np.concatenate([W[:, :, 0].T, W[:, :, 1].T, W[:, :, 2].T,
                         b[None, :]], axis=0)                      # [241,222]
    spack = np.zeros((128, 956), np.float32)
    spack[0:121, 0:BT] = xcat[0:121]
    spack[0:120, BT:2 * BT] = xcat[121:241]
    spack[0:121, 2 * BT:2 * BT + CCEP] = w2[0:121]
    spack[0:120, 2 * BT + CCEP:2 * BT + 2 * CCEP] = w2[121:241]
    zpad = np.concatenate(
        [np.zeros((B, HOP), np.float32), z[:, 0, :]], axis=1)     # [2, 33024]
    shared = {"spack": spack, "zpad": zpad,
              "ident": np.eye(128, dtype=np.float32)}
    in_maps = [{**shared, **_CONSTS[c]} for c in range(NCORES)]

    nc = _get_nc()
    res = run_bass_kernel_spmd(nc, in_maps, list(range(NCORES)), trace=TRACE)
    LAST_RESULT = res
    out = np.zeros((B, 1, T * HOP), dtype=np.float32)
    for r in res.results:
        out += np.asarray(r["out"], dtype=np.float32)
    return out
